# revision 12
# baseline (speedup 1.0000x reference)
"""AttentionHead kernel for 8 Trainium2 NeuronCores.

Problem: x[4,2048,1024] -> Q/K/V projections (qkv_dim=128) -> softmax(Q K^T / sqrt(128)) @ V.

Sharding: core c handles batch b=c//2, query half h=c%2 (1024 queries), with the
full 2048-key sequence for that batch kept local (data-parallel over batch x
query-split; the SxS score matrix stays on-core per the hint). K/V rows are
processed in the order [this core's query half, other half] - softmax and the
attention-weighted sum are permutation-invariant over keys, so each core can
consume the two halves in its own order and no re-indexing is needed.

Per-core pipeline (fp16 compute, fp32 accumulation everywhere):
 1. x rows stream HBM->SBUF through cast-DMAs (SWDGE inline fp32->fp16), then
    XBAR DMA-transposes produce x^T [d,s] - the PE never touches transposes.
 2. Projections contract d in 8 128-chunks: W.T @ x^T accumulated in PSUM
    (fp32), ACT copyback fuses the per-partition bias and rounds to fp16,
    giving Q^T/K^T/V^T in [e, s] layout; one more XBAR turns V^T into natural
    V [k, e].
 3. Attention runs transposed: scores^T[k,q] = K^T-chunk.T @ Q^T; ACT exp
    fuses the 1/sqrt(128) scale (no max subtraction needed - scores ~N(0,1));
    PV accumulates V.T @ expS^T over the 16 k-chunks in PSUM, a parallel
    ones-matmul accumulates the softmax denominators. Normalization happens
    once on the [e,q] accumulator, then 128x128 PE transposes emit [q,e].
"""

import sys

if "/opt/trn_rl_repo" not in sys.path:
    sys.path.insert(0, "/opt/trn_rl_repo")

import numpy as np

P = 128
D = 1024  # d_model
DC = D // P  # 8 contraction chunks
E = 128  # qkv dim
SQ = 1024  # queries per core
SK = 2048  # keys per core
QT = 512  # query column-block width
NQT = SQ // QT  # 2
NKC = SK // P  # 16 key chunks
NST = SK // P  # 16 s-tiles of x
SCALE = 1.0 / float(np.sqrt(E))

_cache: dict = {}

# Set by the first kernel() call; test harnesses can read .exec_time_ns etc.
LAST_RESULT = None


def _build():
    if "nc" in _cache:
        return _cache["nc"]

    import concourse.tile as tile
    from concourse import bacc, mybir
    from concourse.masks import make_identity

    ACTF = mybir.ActivationFunctionType
    f32 = mybir.dt.float32
    f16 = mybir.dt.float16

    nc = bacc.Bacc("TRN2", target_bir_lowering=False, debug=False, num_devices=8)

    xq_d = nc.dram_tensor("xq", [SQ, D], f32, kind="ExternalInput").ap()
    xo_d = nc.dram_tensor("xo", [SQ, D], f32, kind="ExternalInput").ap()
    wq_d = nc.dram_tensor("wq", [D, E], f32, kind="ExternalInput").ap()
    wk_d = nc.dram_tensor("wk", [D, E], f32, kind="ExternalInput").ap()
    wv_d = nc.dram_tensor("wv", [D, E], f32, kind="ExternalInput").ap()
    bq_d = nc.dram_tensor("bq", [E], f32, kind="ExternalInput").ap()
    bk_d = nc.dram_tensor("bk", [E], f32, kind="ExternalInput").ap()
    bv_d = nc.dram_tensor("bv", [E], f32, kind="ExternalInput").ap()
    out_d = nc.dram_tensor("out", [SQ, E], f32, kind="ExternalOutput").ap()

    with tile.TileContext(nc) as tc:
        with (
            tc.tile_pool(name="const", bufs=1) as const,
            tc.tile_pool(name="xload", bufs=6) as xload,
            tc.tile_pool(name="big", bufs=1) as big,
            tc.tile_pool(name="exps", bufs=6) as exps,
            tc.tile_pool(name="misc", bufs=2) as misc,
            tc.tile_pool(name="ptr", bufs=2, space="PSUM") as ptr,
            tc.tile_pool(name="pacc", bufs=3, space="PSUM") as pacc,
            tc.tile_pool(name="po", bufs=2, space="PSUM") as po,
            tc.tile_pool(name="psum_s", bufs=1, space="PSUM") as psum_s,
        ):
            # ---- x loads first (SWDGE cast-DMA fp32->fp16); they gate everything ----
            x16 = []
            for st in range(NST):
                src = xq_d if st < NST // 2 else xo_d
                row0 = (st % (NST // 2)) * P
                xt = xload.tile([P, D], f16, tag="xin")
                nc.gpsimd.dma_start(xt[:], src[row0 : row0 + P, :])
                x16.append(xt)

            # ---- constants ----
            identf = const.tile([P, P], f32)
            make_identity(nc, identf)
            onesf = const.tile([P, 1], f32)
            nc.gpsimd.memset(onesf, 1.0)
            ones = const.tile([P, 1], f16)
            nc.vector.tensor_copy(ones[:], onesf[:])
            w_sb = {}
            for name, wd in (("q", wq_d), ("k", wk_d), ("v", wv_d)):
                wf = const.tile([P, DC, E], f32, name=f"w{name}f")
                nc.scalar.dma_start(wf[:], wd.rearrange("(t p) e -> p t e", p=P))
                w = const.tile([P, DC, E], f16, name=f"w{name}")
                nc.vector.tensor_copy(w[:], wf[:])
                w_sb[name] = w
            b_sb = {}
            for name, bd in (("q", bq_d), ("k", bk_d), ("v", bv_d)):
                b = const.tile([P, 1], f32, name=f"b{name}")
                nc.scalar.dma_start(b[:], bd[:, None])
                b_sb[name] = b

            # ---- big persistent tiles ----
            xT = big.tile([P, DC, SK], f16)  # x^T: [d_lo, d_chunk, s]
            qT = big.tile([P, SQ], f16)  # Q^T: [e, q]
            kT = big.tile([P, SK], f16)  # K^T: [e, k]
            vT = big.tile([P, SK], f16)  # V^T: [e, k] (staging)
            v_sb = big.tile([P, NKC, E], f16)  # V natural: [k_lo, k_chunk, e]

            # ---- phase 1: XBAR DMA-transpose x into xT ----
            for st in range(NST):
                nc.sync.dma_start_transpose(
                    xT[:, :, st * P : (st + 1) * P], x16[st][:]
                )

            # ---- phase 2: projections ----
            def proj(dst_col0, width, w, b, dst):
                psum = pacc.tile([P, QT], f32, tag="mm")
                for dc in range(DC):
                    nc.tensor.matmul(
                        psum[:, :width],
                        w[:, dc, :],
                        xT[:, dc, dst_col0 : dst_col0 + width],
                        start=(dc == 0),
                        stop=(dc == DC - 1),
                    )
                # copyback + per-partition bias (e lives on partitions)
                nc.scalar.activation(
                    dst[:, dst_col0 : dst_col0 + width],
                    psum[:, :width],
                    ACTF.Identity,
                    bias=b[:],
                    scale=1.0,
                )

            for ct in range(SK // QT):
                col0 = ct * QT
                if ct < SQ // QT:
                    proj(col0, QT, w_sb["q"], b_sb["q"], qT)
                proj(col0, QT, w_sb["k"], b_sb["k"], kT)
                proj(col0, QT, w_sb["v"], b_sb["v"], vT)

            # ---- phase 3: V^T -> V natural layout (one XBAR) ----
            nc.sync.dma_start_transpose(v_sb[:], vT[:])

            # ---- phase 4: attention (transposed layout) ----
            for qt in range(NQT):
                q0 = qt * QT
                acc_o = po.tile([P, QT], f32, tag="acc_o")  # out^T accum [e, q]
                acc_s = psum_s.tile([1, QT], f32, tag="acc_s")  # softmax sums
                for kc in range(NKC):
                    ps = pacc.tile([P, QT], f32, tag="mm")
                    nc.tensor.matmul(
                        ps[:],
                        kT[:, kc * P : (kc + 1) * P],
                        qT[:, q0 : q0 + QT],
                        start=True,
                        stop=True,
                    )
                    es = exps.tile([P, QT], f16, tag="exps")
                    nc.scalar.activation(es[:], ps[:], ACTF.Exp, scale=SCALE)
                    nc.tensor.matmul(
                        acc_o[:],
                        v_sb[:, kc, :],
                        es[:],
                        start=(kc == 0),
                        stop=(kc == NKC - 1),
                    )
                    nc.tensor.matmul(
                        acc_s[:],
                        ones[:],
                        es[:],
                        start=(kc == 0),
                        stop=(kc == NKC - 1),
                    )
                # normalize: out^T[e, q] / sums[q]
                sums_sb = misc.tile([1, QT], f32, tag="sums")
                nc.vector.tensor_copy(sums_sb[:], acc_s[:])
                recip = misc.tile([1, QT], f32, tag="recip")
                nc.vector.reciprocal_approx_fast(recip[:], sums_sb[:])
                rbc = misc.tile([P, QT], f32, tag="rbc")
                nc.gpsimd.partition_broadcast(rbc[:], recip[:])
                otn = misc.tile([P, QT], f32, tag="otn")
                nc.vector.tensor_mul(out=otn[:], in0=acc_o[:], in1=rbc[:])
                # transpose back to [q, e] and store
                out_sb = misc.tile([P, QT // P, E], f32, tag="outsb")
                for j in range(QT // P):
                    ps = ptr.tile([P, P], f32, tag="tr")
                    nc.tensor.transpose(ps[:], otn[:, j * P : (j + 1) * P], identf[:])
                    nc.vector.tensor_copy(out_sb[:, j, :], ps[:])
                nc.sync.dma_start(
                    out_d[q0 : q0 + QT, :].rearrange("(t p) e -> p t e", p=P),
                    out_sb[:],
                )

    nc.compile()
    _cache["nc"] = nc
    return nc


def kernel(x, Wq, bq, Wk, bk, Wv, bv):
    global LAST_RESULT
    nc = _build()
    from concourse import bass_utils

    x = np.asarray(x, dtype=np.float32)
    Wq = np.ascontiguousarray(np.asarray(Wq, dtype=np.float32))
    Wk = np.ascontiguousarray(np.asarray(Wk, dtype=np.float32))
    Wv = np.ascontiguousarray(np.asarray(Wv, dtype=np.float32))
    bq = np.ascontiguousarray(np.asarray(bq, dtype=np.float32))
    bk = np.ascontiguousarray(np.asarray(bk, dtype=np.float32))
    bv = np.ascontiguousarray(np.asarray(bv, dtype=np.float32))
    B, S, _ = x.shape

    in_maps = []
    for c in range(8):
        b, h = c // 2, c % 2
        xq = np.ascontiguousarray(x[b, h * SQ : (h + 1) * SQ])
        xo = np.ascontiguousarray(x[b, (1 - h) * SQ : (2 - h) * SQ])
        in_maps.append(
            {
                "xq": xq,
                "xo": xo,
                "wq": Wq,
                "wk": Wk,
                "wv": Wv,
                "bq": bq,
                "bk": bk,
                "bv": bv,
            }
        )

    res = bass_utils.run_bass_kernel_spmd(nc, in_maps, core_ids=list(range(8)))
    LAST_RESULT = res

    out = np.empty((B, S, E), dtype=np.float32)
    for c in range(8):
        b, h = c // 2, c % 2
        out[b, h * SQ : (h + 1) * SQ] = res.results[c]["out"]
    return out


# revision 17
# speedup vs baseline: 1.3462x; 1.3462x over previous
"""AttentionHead kernel for 8 Trainium2 NeuronCores.

Problem: x[4,2048,1024] -> Q/K/V projections (qkv_dim=128) -> softmax(Q K^T / sqrt(128)) @ V.

Sharding: core c handles batch b=c//2, query half h=c%2 (1024 queries), with the
full 2048-key sequence for that batch kept local (data-parallel over batch x
query-split; the SxS score matrix stays on-core per the hint). K/V rows are
processed in the order [this core's query half, other half] - softmax and the
attention-weighted sum are permutation-invariant over keys, so each core can
consume the two halves in its own order and no re-indexing is needed.

Per-core pipeline (fp16 compute, fp32 accumulation everywhere):
 1. x rows stream HBM->SBUF through cast-DMAs (SWDGE inline fp32->fp16), then
    XBAR DMA-transposes produce x^T [d,s] - the PE never touches transposes.
 2. Projections contract d in 8 128-chunks: W.T @ x^T accumulated in PSUM
    (fp32), ACT copyback fuses the per-partition bias and rounds to fp16,
    giving Q^T/K^T/V^T in [e, s] layout; one more XBAR turns V^T into natural
    V [k, e].
 3. Attention runs transposed: scores^T[k,q] = K^T-chunk.T @ Q^T; ACT exp
    fuses the 1/sqrt(128) scale (no max subtraction needed - scores ~N(0,1));
    PV accumulates V.T @ expS^T over the 16 k-chunks in PSUM, a parallel
    ones-matmul accumulates the softmax denominators. Normalization happens
    once on the [e,q] accumulator, then 128x128 PE transposes emit [q,e].
"""

import sys

if "/opt/trn_rl_repo" not in sys.path:
    sys.path.insert(0, "/opt/trn_rl_repo")

import numpy as np

P = 128
D = 1024  # d_model
DC = D // P  # 8 contraction chunks
E = 128  # qkv dim
SQ = 1024  # queries per core
SK = 2048  # keys per core
QT = 512  # query column-block width
NQT = SQ // QT  # 2
NKC = SK // P  # 16 key chunks
NST = SK // P  # 16 s-tiles of x
SCALE = 1.0 / float(np.sqrt(E))

_cache: dict = {}

# Set by the first kernel() call; test harnesses can read .exec_time_ns etc.
LAST_RESULT = None


def _build():
    if "nc" in _cache:
        return _cache["nc"]

    import concourse.tile as tile
    from concourse import bacc, mybir
    from concourse.masks import make_identity

    ACTF = mybir.ActivationFunctionType
    f32 = mybir.dt.float32
    f16 = mybir.dt.float16

    nc = bacc.Bacc("TRN2", target_bir_lowering=False, debug=False, num_devices=8)

    xq_d = nc.dram_tensor("xq", [SQ, D], f32, kind="ExternalInput").ap()
    xo_d = nc.dram_tensor("xo", [SQ, D], f32, kind="ExternalInput").ap()
    wq_d = nc.dram_tensor("wq", [D, E], f32, kind="ExternalInput").ap()
    wk_d = nc.dram_tensor("wk", [D, E], f32, kind="ExternalInput").ap()
    wv_d = nc.dram_tensor("wv", [D, E], f32, kind="ExternalInput").ap()
    bq_d = nc.dram_tensor("bq", [E], f32, kind="ExternalInput").ap()
    bk_d = nc.dram_tensor("bk", [E], f32, kind="ExternalInput").ap()
    bv_d = nc.dram_tensor("bv", [E], f32, kind="ExternalInput").ap()
    out_d = nc.dram_tensor("out", [SQ, E], f32, kind="ExternalOutput").ap()

    with tile.TileContext(nc) as tc:
        with (
            tc.tile_pool(name="const", bufs=1) as const,
            tc.tile_pool(name="xload", bufs=16) as xload,
            tc.tile_pool(name="big", bufs=1) as big,
            tc.tile_pool(name="exps", bufs=8) as exps,
            tc.tile_pool(name="misc", bufs=2) as misc,
            tc.tile_pool(name="ptr", bufs=2, space="PSUM") as ptr,
            tc.tile_pool(name="pacc", bufs=3, space="PSUM") as pacc,
            tc.tile_pool(name="po", bufs=2, space="PSUM") as po,
            tc.tile_pool(name="psum_s", bufs=1, space="PSUM") as psum_s,
        ):
            # ---- x loads first (SWDGE cast-DMA fp32->fp16); they gate everything ----
            x16 = []
            copy_dmas = []
            for st in range(NST):
                src = xq_d if st < NST // 2 else xo_d
                row0 = (st % (NST // 2)) * P
                xt = xload.tile([P, D], f16, tag="xin")
                copy_dmas.append(nc.gpsimd.dma_start(xt[:], src[row0 : row0 + P, :]))
                x16.append(xt)

            # ---- constants ----
            identf = const.tile([P, P], f32)
            make_identity(nc, identf)
            onesf = const.tile([P, 1], f32)
            nc.gpsimd.memset(onesf, 1.0)
            w_sb = {}
            for name, wd in (("q", wq_d), ("k", wk_d), ("v", wv_d)):
                wf = const.tile([P, DC, E], f32, name=f"w{name}f")
                copy_dmas.append(
                    nc.gpsimd.dma_start(wf[:], wd.rearrange("(t p) e -> p t e", p=P))
                )
                w = const.tile([P, DC, E], f16, name=f"w{name}")
                nc.vector.tensor_copy(w[:], wf[:])
                w_sb[name] = w
            b_sb = {}
            for name, bd in (("q", bq_d), ("k", bk_d), ("v", bv_d)):
                b = const.tile([P, 1], f32, name=f"b{name}")
                copy_dmas.append(nc.gpsimd.dma_start(b[:], bd[:, None]))
                b_sb[name] = b

            # ---- big persistent tiles ----
            xT = big.tile([P, DC, SK], f16)  # x^T: [d_lo, d_chunk, s]
            qT = big.tile([P, SQ], f16)  # Q^T: [e, q]
            kT = big.tile([P, SK], f16)  # K^T: [e, k]
            vT = big.tile([P, SK], f16)  # V^T: [e, k] (staging)
            v_sb = big.tile([P, NKC, E], f16)  # V natural: [k_lo, k_chunk, e]

            # ---- phase 1: XBAR DMA-transpose x into xT ----
            # All copy-DMAs must complete before any XBAR transpose: mixing
            # the two DMA modes makes Tile serialize every transition (HW
            # xbar-mode hazard), which costs ~3us per flip.
            from concourse.tile import add_dep_helper

            barrier = [copy_dmas[-1], copy_dmas[NST - 1]]
            for st in range(NST):
                xb = nc.sync.dma_start_transpose(
                    xT[:, :, st * P : (st + 1) * P], x16[st][:]
                )
                for bdep in barrier:
                    add_dep_helper(xb.ins, bdep.ins, reason="copies before xbars")

            # ---- phase 2: projections ----
            def proj(dst_col0, width, w, b, dst):
                psum = pacc.tile([P, QT], f32, tag="mm")
                for dc in range(DC):
                    nc.tensor.matmul(
                        psum[:, :width],
                        w[:, dc, :],
                        xT[:, dc, dst_col0 : dst_col0 + width],
                        start=(dc == 0),
                        stop=(dc == DC - 1),
                    )
                # copyback + per-partition bias (e lives on partitions)
                nc.scalar.activation(
                    dst[:, dst_col0 : dst_col0 + width],
                    psum[:, :width],
                    ACTF.Identity,
                    bias=b[:],
                    scale=1.0,
                )

            for ct in range(SK // QT):
                col0 = ct * QT
                if ct < SQ // QT:
                    proj(col0, QT, w_sb["q"], b_sb["q"], qT)
                proj(col0, QT, w_sb["k"], b_sb["k"], kT)
                proj(col0, QT, w_sb["v"], b_sb["v"], vT)

            # ---- phase 3: V^T -> V natural layout (one XBAR) ----
            nc.sync.dma_start_transpose(v_sb[:], vT[:])

            # ---- phase 4: attention (transposed layout) ----
            for qt in range(NQT):
                q0 = qt * QT
                acc_o = po.tile([P, QT], f32, tag="acc_o")  # out^T accum [e, q]
                asum = misc.tile([P, QT], f32, tag="asum")  # partial exp sums
                for kc in range(NKC):
                    ps = pacc.tile([P, QT], f32, tag="mm")
                    nc.tensor.matmul(
                        ps[:],
                        kT[:, kc * P : (kc + 1) * P],
                        qT[:, q0 : q0 + QT],
                        start=True,
                        stop=True,
                    )
                    es = exps.tile([P, QT], f16, tag="exps")
                    nc.scalar.activation(es[:], ps[:], ACTF.Exp, scale=SCALE)
                    nc.tensor.matmul(
                        acc_o[:],
                        v_sb[:, kc, :],
                        es[:],
                        start=(kc == 0),
                        stop=(kc == NKC - 1),
                    )
                    if kc == 0:
                        nc.vector.tensor_copy(asum[:], es[:])
                    else:
                        nc.vector.tensor_add(out=asum[:], in0=asum[:], in1=es[:])
                # cross-partition reduce of the exp sums (fp32 matmul, N=512)
                acc_s = psum_s.tile([1, QT], f32, tag="acc_s")
                nc.tensor.matmul(acc_s[:], onesf[:], asum[:], start=True, stop=True)
                # normalize: out^T[e, q] / sums[q]
                sums_sb = misc.tile([1, QT], f32, tag="sums")
                nc.vector.tensor_copy(sums_sb[:], acc_s[:])
                recip = misc.tile([1, QT], f32, tag="recip")
                nc.vector.reciprocal_approx_fast(recip[:], sums_sb[:])
                rbc = misc.tile([P, QT], f32, tag="rbc")
                nc.gpsimd.partition_broadcast(rbc[:], recip[:])
                otn = misc.tile([P, QT], f32, tag="otn")
                nc.vector.tensor_mul(out=otn[:], in0=acc_o[:], in1=rbc[:])
                # transpose back to [q, e] and store
                out_sb = misc.tile([P, QT // P, E], f32, tag="outsb")
                for j in range(QT // P):
                    ps = ptr.tile([P, P], f32, tag="tr")
                    nc.tensor.transpose(ps[:], otn[:, j * P : (j + 1) * P], identf[:])
                    nc.vector.tensor_copy(out_sb[:, j, :], ps[:])
                nc.sync.dma_start(
                    out_d[q0 : q0 + QT, :].rearrange("(t p) e -> p t e", p=P),
                    out_sb[:],
                )

    nc.compile()
    _cache["nc"] = nc
    return nc


def kernel(x, Wq, bq, Wk, bk, Wv, bv):
    global LAST_RESULT
    nc = _build()
    from concourse import bass_utils

    x = np.asarray(x, dtype=np.float32)
    Wq = np.ascontiguousarray(np.asarray(Wq, dtype=np.float32))
    Wk = np.ascontiguousarray(np.asarray(Wk, dtype=np.float32))
    Wv = np.ascontiguousarray(np.asarray(Wv, dtype=np.float32))
    bq = np.ascontiguousarray(np.asarray(bq, dtype=np.float32))
    bk = np.ascontiguousarray(np.asarray(bk, dtype=np.float32))
    bv = np.ascontiguousarray(np.asarray(bv, dtype=np.float32))
    B, S, _ = x.shape

    in_maps = []
    for c in range(8):
        b, h = c // 2, c % 2
        xq = np.ascontiguousarray(x[b, h * SQ : (h + 1) * SQ])
        xo = np.ascontiguousarray(x[b, (1 - h) * SQ : (2 - h) * SQ])
        in_maps.append(
            {
                "xq": xq,
                "xo": xo,
                "wq": Wq,
                "wk": Wk,
                "wv": Wv,
                "bq": bq,
                "bk": bk,
                "bv": bv,
            }
        )

    res = bass_utils.run_bass_kernel_spmd(nc, in_maps, core_ids=list(range(8)))
    LAST_RESULT = res

    out = np.empty((B, S, E), dtype=np.float32)
    for c in range(8):
        b, h = c // 2, c % 2
        out[b, h * SQ : (h + 1) * SQ] = res.results[c]["out"]
    return out


# revision 19
# speedup vs baseline: 1.4957x; 1.1111x over previous
"""AttentionHead kernel for 8 Trainium2 NeuronCores.

Problem: x[4,2048,1024] -> Q/K/V projections (qkv_dim=128) -> softmax(Q K^T / sqrt(128)) @ V.

Sharding: core c handles batch b=c//2, query half h=c%2 (1024 queries), with the
full 2048-key sequence for that batch kept local (data-parallel over batch x
query-split; the SxS score matrix stays on-core per the hint). K/V rows are
processed in the order [this core's query half, other half] - softmax and the
attention-weighted sum are permutation-invariant over keys, so each core can
consume the two halves in its own order and no re-indexing is needed.

Per-core pipeline (fp16 compute, fp32 accumulation everywhere):
 1. x rows stream HBM->SBUF through cast-DMAs (SWDGE inline fp32->fp16), then
    XBAR DMA-transposes produce x^T [d,s] - the PE never touches transposes.
 2. Projections contract d in 8 128-chunks: W.T @ x^T accumulated in PSUM
    (fp32), ACT copyback fuses the per-partition bias and rounds to fp16,
    giving Q^T/K^T/V^T in [e, s] layout; one more XBAR turns V^T into natural
    V [k, e].
 3. Attention runs transposed: scores^T[k,q] = K^T-chunk.T @ Q^T; ACT exp
    fuses the 1/sqrt(128) scale (no max subtraction needed - scores ~N(0,1));
    PV accumulates V.T @ expS^T over the 16 k-chunks in PSUM, a parallel
    ones-matmul accumulates the softmax denominators. Normalization happens
    once on the [e,q] accumulator, then 128x128 PE transposes emit [q,e].
"""

import sys

if "/opt/trn_rl_repo" not in sys.path:
    sys.path.insert(0, "/opt/trn_rl_repo")

import numpy as np

P = 128
D = 1024  # d_model
DC = D // P  # 8 contraction chunks
E = 128  # qkv dim
SQ = 1024  # queries per core
SK = 2048  # keys per core
QT = 512  # query column-block width
NQT = SQ // QT  # 2
NKC = SK // P  # 16 key chunks
NST = SK // P  # 16 s-tiles of x
SCALE = 1.0 / float(np.sqrt(E))

_cache: dict = {}

# Set by the first kernel() call; test harnesses can read .exec_time_ns etc.
LAST_RESULT = None


def _build():
    if "nc" in _cache:
        return _cache["nc"]

    import concourse.tile as tile
    from concourse import bacc, mybir
    from concourse.masks import make_identity

    ACTF = mybir.ActivationFunctionType
    f32 = mybir.dt.float32
    f16 = mybir.dt.float16

    nc = bacc.Bacc("TRN2", target_bir_lowering=False, debug=False, num_devices=8)

    xq_d = nc.dram_tensor("xq", [SQ, D], f32, kind="ExternalInput").ap()
    xo_d = nc.dram_tensor("xo", [SQ, D], f32, kind="ExternalInput").ap()
    wq_d = nc.dram_tensor("wq", [D, E], f32, kind="ExternalInput").ap()
    wk_d = nc.dram_tensor("wk", [D, E], f32, kind="ExternalInput").ap()
    wv_d = nc.dram_tensor("wv", [D, E], f32, kind="ExternalInput").ap()
    bq_d = nc.dram_tensor("bq", [E], f32, kind="ExternalInput").ap()
    bk_d = nc.dram_tensor("bk", [E], f32, kind="ExternalInput").ap()
    bv_d = nc.dram_tensor("bv", [E], f32, kind="ExternalInput").ap()
    out_d = nc.dram_tensor("out", [SQ, E], f32, kind="ExternalOutput").ap()

    with tile.TileContext(nc) as tc:
        with (
            tc.tile_pool(name="const", bufs=1) as const,
            tc.tile_pool(name="xload", bufs=16) as xload,
            tc.tile_pool(name="big", bufs=1) as big,
            tc.tile_pool(name="exps", bufs=8) as exps,
            tc.tile_pool(name="misc", bufs=2) as misc,
            tc.tile_pool(name="ptr", bufs=2, space="PSUM") as ptr,
            tc.tile_pool(name="pacc", bufs=3, space="PSUM") as pacc,
            tc.tile_pool(name="po", bufs=2, space="PSUM") as po,
            tc.tile_pool(name="psum_s", bufs=1, space="PSUM") as psum_s,
        ):
            # ---- constants first: small loads that unblock the projections ----
            copy_dmas = []
            identf = const.tile([P, P], f32)
            make_identity(nc, identf)
            ident16 = const.tile([P, P], f16)
            nc.vector.tensor_copy(ident16[:], identf[:])
            onesf = const.tile([P, 1], f32)
            nc.gpsimd.memset(onesf, 1.0)
            w_sb = {}
            for name, wd in (("q", wq_d), ("k", wk_d), ("v", wv_d)):
                wf = const.tile([P, DC, E], f32, name=f"w{name}f")
                copy_dmas.append(
                    nc.gpsimd.dma_start(wf[:], wd.rearrange("(t p) e -> p t e", p=P))
                )
                w = const.tile([P, DC, E], f16, name=f"w{name}")
                nc.vector.tensor_copy(w[:], wf[:])
                w_sb[name] = w
            b_sb = {}
            for name, bd in (("q", bq_d), ("k", bk_d), ("v", bv_d)):
                b = const.tile([P, 1], f32, name=f"b{name}")
                copy_dmas.append(nc.gpsimd.dma_start(b[:], bd[:, None]))
                b_sb[name] = b

            # ---- x loads (SWDGE cast-DMA fp32->fp16) ----
            x16 = []
            for st in range(NST):
                src = xq_d if st < NST // 2 else xo_d
                row0 = (st % (NST // 2)) * P
                xt = xload.tile([P, D], f16, tag="xin")
                copy_dmas.append(nc.gpsimd.dma_start(xt[:], src[row0 : row0 + P, :]))
                x16.append(xt)

            # ---- big persistent tiles ----
            xT = big.tile([P, DC, SK], f16)  # x^T: [d_lo, d_chunk, s]
            qT = big.tile([P, SQ], f16)  # Q^T: [e, q]
            kT = big.tile([P, SK], f16)  # K^T: [e, k]
            vT = big.tile([P, SK], f16)  # V^T: [e, k] (staging)
            v_sb = big.tile([P, NKC, E], f16)  # V natural: [k_lo, k_chunk, e]

            # ---- phase 1: transpose x into xT ----
            # First half (this core's query rows) via PE transposes so the PE
            # has work immediately; second half via XBAR DMA-transposes, which
            # must all run after every copy-DMA completes (mixing the two DMA
            # modes makes Tile serialize every transition - HW xbar hazard).
            from concourse.tile import add_dep_helper

            NPE = NST // 2
            for st in range(NPE):
                for dc in range(DC):
                    ps = ptr.tile([P, P], f16, tag="tr")
                    nc.tensor.transpose(
                        ps[:], x16[st][:, dc * P : (dc + 1) * P], ident16[:]
                    )
                    dst = xT[:, dc, st * P : (st + 1) * P]
                    if (st + dc) % 2 == 0:
                        nc.vector.tensor_copy(dst, ps[:])
                    else:
                        nc.scalar.activation(dst, ps[:], ACTF.Copy)

            barrier = copy_dmas[-1]
            for st in range(NPE, NST):
                xb = nc.sync.dma_start_transpose(
                    xT[:, :, st * P : (st + 1) * P], x16[st][:]
                )
                add_dep_helper(xb.ins, barrier.ins, reason="copies before xbars")

            # ---- phase 2: projections ----
            def proj(dst_col0, width, w, b, dst):
                psum = pacc.tile([P, QT], f32, tag="mm")
                for dc in range(DC):
                    nc.tensor.matmul(
                        psum[:, :width],
                        w[:, dc, :],
                        xT[:, dc, dst_col0 : dst_col0 + width],
                        start=(dc == 0),
                        stop=(dc == DC - 1),
                    )
                # copyback + per-partition bias (e lives on partitions)
                nc.scalar.activation(
                    dst[:, dst_col0 : dst_col0 + width],
                    psum[:, :width],
                    ACTF.Identity,
                    bias=b[:],
                    scale=1.0,
                )

            for ct in range(SK // QT):
                col0 = ct * QT
                if ct < SQ // QT:
                    proj(col0, QT, w_sb["q"], b_sb["q"], qT)
                proj(col0, QT, w_sb["k"], b_sb["k"], kT)
                proj(col0, QT, w_sb["v"], b_sb["v"], vT)

            # ---- phase 3: V^T -> V natural layout (one XBAR) ----
            nc.sync.dma_start_transpose(v_sb[:], vT[:])

            # ---- phase 4: attention (transposed layout) ----
            for qt in range(NQT):
                q0 = qt * QT
                acc_o = po.tile([P, QT], f32, tag="acc_o")  # out^T accum [e, q]
                asum = misc.tile([P, QT], f32, tag="asum")  # partial exp sums
                for kc in range(NKC):
                    ps = pacc.tile([P, QT], f32, tag="mm")
                    nc.tensor.matmul(
                        ps[:],
                        kT[:, kc * P : (kc + 1) * P],
                        qT[:, q0 : q0 + QT],
                        start=True,
                        stop=True,
                    )
                    es = exps.tile([P, QT], f16, tag="exps")
                    nc.scalar.activation(es[:], ps[:], ACTF.Exp, scale=SCALE)
                    nc.tensor.matmul(
                        acc_o[:],
                        v_sb[:, kc, :],
                        es[:],
                        start=(kc == 0),
                        stop=(kc == NKC - 1),
                    )
                    if kc == 0:
                        nc.vector.tensor_copy(asum[:], es[:])
                    else:
                        nc.vector.tensor_add(out=asum[:], in0=asum[:], in1=es[:])
                # cross-partition reduce of the exp sums (fp32 matmul, N=512)
                acc_s = psum_s.tile([1, QT], f32, tag="acc_s")
                nc.tensor.matmul(acc_s[:], onesf[:], asum[:], start=True, stop=True)
                # normalize: out^T[e, q] / sums[q]
                sums_sb = misc.tile([1, QT], f32, tag="sums")
                nc.vector.tensor_copy(sums_sb[:], acc_s[:])
                recip = misc.tile([1, QT], f32, tag="recip")
                nc.vector.reciprocal_approx_fast(recip[:], sums_sb[:])
                rbc = misc.tile([P, QT], f32, tag="rbc")
                nc.gpsimd.partition_broadcast(rbc[:], recip[:])
                otn = misc.tile([P, QT], f32, tag="otn")
                nc.vector.tensor_mul(out=otn[:], in0=acc_o[:], in1=rbc[:])
                # transpose back to [q, e] and store
                out_sb = misc.tile([P, QT // P, E], f32, tag="outsb")
                for j in range(QT // P):
                    ps = ptr.tile([P, P], f32, tag="tr")
                    nc.tensor.transpose(ps[:], otn[:, j * P : (j + 1) * P], identf[:])
                    nc.vector.tensor_copy(out_sb[:, j, :], ps[:])
                nc.sync.dma_start(
                    out_d[q0 : q0 + QT, :].rearrange("(t p) e -> p t e", p=P),
                    out_sb[:],
                )

    nc.compile()
    _cache["nc"] = nc
    return nc


def kernel(x, Wq, bq, Wk, bk, Wv, bv):
    global LAST_RESULT
    nc = _build()
    from concourse import bass_utils

    x = np.asarray(x, dtype=np.float32)
    Wq = np.ascontiguousarray(np.asarray(Wq, dtype=np.float32))
    Wk = np.ascontiguousarray(np.asarray(Wk, dtype=np.float32))
    Wv = np.ascontiguousarray(np.asarray(Wv, dtype=np.float32))
    bq = np.ascontiguousarray(np.asarray(bq, dtype=np.float32))
    bk = np.ascontiguousarray(np.asarray(bk, dtype=np.float32))
    bv = np.ascontiguousarray(np.asarray(bv, dtype=np.float32))
    B, S, _ = x.shape

    in_maps = []
    for c in range(8):
        b, h = c // 2, c % 2
        xq = np.ascontiguousarray(x[b, h * SQ : (h + 1) * SQ])
        xo = np.ascontiguousarray(x[b, (1 - h) * SQ : (2 - h) * SQ])
        in_maps.append(
            {
                "xq": xq,
                "xo": xo,
                "wq": Wq,
                "wk": Wk,
                "wv": Wv,
                "bq": bq,
                "bk": bk,
                "bv": bv,
            }
        )

    res = bass_utils.run_bass_kernel_spmd(nc, in_maps, core_ids=list(range(8)))
    LAST_RESULT = res

    out = np.empty((B, S, E), dtype=np.float32)
    for c in range(8):
        b, h = c // 2, c % 2
        out[b, h * SQ : (h + 1) * SQ] = res.results[c]["out"]
    return out
